# revision 17
# baseline (speedup 1.0000x reference)
"""Distributed 3-layer GAT (BioGNN) for 8 TRN2 NeuronCores.

Sharding: nodes partitioned into 8 contiguous shards of 2500; each core owns
the edges whose dst lies in its shard (dst-sorted, grouped into 32-dst
windows, padded to 128-edge tiles; per-window tile counts padded to the max
across cores so all cores run one SPMD program). Per layer: a local dense
matmul produces a node-feature table shard ([h|1|...|al_s] rows) which is
AllGathered to every core; the edge phase dma_gathers source rows per edge,
computes softmax weights on DVE/ACT, and aggregates per dst-window with
selection-matrix matmuls accumulating in PSUM. Readout is a batch-one-hot
matmul, AllReduced, finished with a small on-chip MLP.

Host-side work is integer index preprocessing only; all float math runs on
the NeuronCores.
"""
import sys
sys.path.insert(0, "/opt/trn_rl_repo")
import numpy as np

NC = 8
N = 20000
SHN = N // NC            # 2500
FIN = 512
HID = 64
NGR = 64
NCLS = 5
P = 128
W = 32                   # dst window
KCH = 8                  # tiles per gather chunk (1024-idx gathers: larger crashes on Shared tables)
ROW1 = 320               # layer1/2 table row floats
ROW3 = 128               # layer3 table row floats
SLOPE = 0.2
EPS = 1e-5
BF16_TABLE = False       # table/S/gather/hh in bf16 (2x gather+exchange, ~2.4e-4 rel err)
NCHK = (SHN + P - 1) // P                    # 20 node chunks per core
NWIN = ((SHN + W - 1) // W + 3) // 4 * 4     # 80 windows


def _preprocess(edge_index):
    """Index-only host work. Uniform program: per-window tile counts padded to
    the max across cores. Returns per-core arrays + global tile->window map."""
    src = np.concatenate([edge_index[0].astype(np.int64), np.arange(N, dtype=np.int64)])
    dst = np.concatenate([edge_index[1].astype(np.int64), np.arange(N, dtype=np.int64)])
    per_core = []
    for c in range(NC):
        lo = c * SHN
        m = (dst >= lo) & (dst < lo + SHN)
        es, ed = src[m], dst[m] - lo
        order = np.argsort(ed, kind="stable")
        es, ed = es[order], ed[order]
        wins = []
        for w in range(NWIN):
            wm = (ed >= w * W) & (ed < (w + 1) * W)
            ws, wd = es[wm], (ed[wm] - w * W)
            # fake dst slots (beyond SHN) get one S=1 edge so denom != 0
            fake_lo = max(0, min(W, SHN - w * W))
            nfk = W - fake_lo
            ws = np.concatenate([ws, np.zeros(nfk, np.int64)])
            wd = np.concatenate([wd, np.arange(fake_lo, W, dtype=np.int64)])
            val = np.ones(len(ws), np.float32)
            wins.append((ws, wd, val))
        per_core.append(wins)

    tiles_per_win = []
    for w in range(NWIN):
        mx = max(len(per_core[c][w][0]) for c in range(NC))
        tiles_per_win.append(max(1, (mx + P - 1) // P))
    T = sum(tiles_per_win)
    win_of_tile = []
    for w, nt in enumerate(tiles_per_win):
        win_of_tile += [w] * nt

    nch = (T + KCH - 1) // KCH
    Tpad = nch * KCH
    core_arrays = []
    for c in range(NC):
        srcs = np.zeros((Tpad, P), np.int64)
        dsts = np.zeros((Tpad, P), np.int64)
        Sarr = np.zeros((Tpad, P, W), np.float32)
        t0 = 0
        for w, nt in enumerate(tiles_per_win):
            ws, wd, val = per_core[c][w]
            ne = len(ws)
            pad = nt * P - ne
            ws = np.concatenate([ws, np.zeros(pad, np.int64)])
            wd = np.concatenate([wd, np.zeros(pad, np.int64)])
            val = np.concatenate([val, np.zeros(pad, np.float32)])
            for t in range(nt):
                sl = slice(t * P, (t + 1) * P)
                srcs[t0 + t] = ws[sl]
                dloc = w * W + wd[sl]
                dsts[t0 + t] = np.where(dloc < SHN, dloc, 0)
                Sarr[t0 + t][np.arange(P), wd[sl]] = val[sl]
            t0 += nt
        # dma_gather idx: per chunk [128, KCH*8] int16; idx position i=j*128+p
        # -> wrapped [i%16, i//16], replicated x8 over the 8 partition groups
        flat = srcs.reshape(nch, KCH * P).astype(np.int16)
        si = np.zeros((nch, P, KCH * 8), np.int16)
        for ch in range(nch):
            w16 = flat[ch].reshape(KCH * 8, 16).T           # [16, KCH*8]
            si[ch] = np.tile(w16, (8, 1))
        # indirect al_d idx: per chunk [128, KCH] int32; [p, j] = local dst of
        # (tile j, partition p); consumed partition-major (p*KCH + j)
        di = dsts.reshape(nch, KCH, P).transpose(0, 2, 1).astype(np.int32)
        Sc = Sarr.reshape(nch, KCH, P, W).transpose(0, 2, 1, 3).reshape(nch, P, KCH * W)
        core_arrays.append(dict(si=np.ascontiguousarray(si),
                                di=np.ascontiguousarray(di),
                                S=np.ascontiguousarray(Sc)))
    return core_arrays, win_of_tile, tiles_per_win, nch


def _batch_S(batch):
    out = []
    for c in range(NC):
        lo = c * SHN
        Sb = np.zeros((NCHK, P, NGR), np.float32)
        for i in range(NCHK):
            b = batch[lo + i * P: min(lo + (i + 1) * P, lo + SHN)]
            Sb[i][np.arange(len(b)), b] = 1.0
        out.append(Sb)
    return out


def _build(win_of_tile, nch, phases=5):
    import concourse.bass as bass
    import concourse.bacc as bacc
    import concourse.mybir as mybir
    import concourse.tile as tile
    from concourse.masks import make_identity
    from contextlib import ExitStack

    f32 = mybir.dt.float32
    tdt = mybir.dt.bfloat16 if BF16_TABLE else f32
    i16 = mybir.dt.int16
    i32 = mybir.dt.int32
    AT = mybir.AluOpType
    ACTF = mybir.ActivationFunctionType
    T = len(win_of_tile)
    win_start = {}
    win_end = {}
    for t, w in enumerate(win_of_tile):
        win_start.setdefault(w, t)
        win_end[w] = t

    nc = bacc.Bacc(num_devices=NC)
    dp = nc.declare_dram_parameter
    x_in = dp("x_sh", [SHN, FIN], f32, isOutput=False)
    W1_in = dp("W1", [FIN, 256], f32, isOutput=False)
    W2_in = dp("W2", [256, 256], f32, isOutput=False)
    W3_in = dp("W3", [256, 64], f32, isOutput=False)
    a_in = {}
    for l, h in ((1, 4), (2, 4), (3, 1)):
        a_in[l, "s"] = dp(f"a{l}s", [1, h * 64], f32, isOutput=False)
        a_in[l, "d"] = dp(f"a{l}d", [1, h * 64], f32, isOutput=False)
    row_in = {}
    for nm in ("b1", "b2", "bn1_gamma", "bn1_beta", "bn1_mean", "bn1_var",
               "bn2_gamma", "bn2_beta", "bn2_mean", "bn2_var"):
        row_in[nm] = dp(nm, [1, 256], f32, isOutput=False)
    b3_in = dp("b3", [1, 64], f32, isOutput=False)
    l1w_in = dp("lin1_w", [64, 32], f32, isOutput=False)
    l1b_in = dp("lin1_b", [1, 32], f32, isOutput=False)
    l2w_in = dp("lin2_w", [32, 5], f32, isOutput=False)
    l2b_in = dp("lin2_b", [1, 5], f32, isOutput=False)
    si_in = dp("si", [nch, P, KCH * 8], i16, isOutput=False)
    di_in = dp("di", [nch, P, KCH], i32, isOutput=False)
    S_in = dp("S", [nch, P, KCH * W], tdt, isOutput=False)
    Sb_in = dp("Sb", [NCHK, P, NGR], f32, isOutput=False)
    out_ext = dp("out", [NGR, NCLS], f32, isOutput=True)
    dbg_ext = dp("dbg", [P, ROW1], f32, isOutput=True)

    shard12 = nc.dram_tensor("shard12", [SHN, ROW1], tdt)
    table12 = nc.dram_tensor("table12", [N, ROW1], tdt, addr_space="Shared")
    shard3 = nc.dram_tensor("shard3", [SHN, ROW3], tdt)
    table3 = nc.dram_tensor("table3", [N, ROW3], tdt, addr_space="Shared")
    ald12 = nc.dram_tensor("ald12", [SHN, 4], f32)
    ald3 = nc.dram_tensor("ald3", [SHN, 1], f32)
    ro_in = nc.dram_tensor("ro_in", [NGR, HID + 1], f32)
    ro_out = nc.dram_tensor("ro_out", [NGR, HID + 1], f32, addr_space="Shared")

    es = ExitStack()
    sbt = lambda name, shape, dt=f32: es.enter_context(nc.sbuf_tensor(name, shape, dt))
    ident = sbt("ident", [P, P])
    ones_row = sbt("ones_row", [1, P])
    W1e = sbt("W1e", [P, 4, 264])
    W2e = sbt("W2e", [P, 2, 264])
    W3e = sbt("W3e", [P, 2, 66])
    Brep = {1: sbt("B1rep", [P, 256]), 2: sbt("B2rep", [P, 256]),
            3: sbt("B3rep", [P, 64])}
    xnext = sbt("xnext", [P, NCHK, 256])
    hfext = sbt("hfext", [P, NCHK, HID + 1])
    lin1e = sbt("lin1e", [65, 32])
    lin2e = sbt("lin2e", [33, 5])
    cc_sems = [es.enter_context(nc.semaphore(f"cc{i}")) for i in range(4)]

    LCFG = {
        1: dict(heads=4, F=FIN, We=W1e, row=ROW1, hhc=260, alc=260,
                shard=shard12, table=table12, ald=ald12),
        2: dict(heads=4, F=256, We=W2e, row=ROW1, hhc=260, alc=260,
                shard=shard12, table=table12, ald=ald12),
        3: dict(heads=1, F=256, We=W3e, row=ROW3, hhc=65, alc=65,
                shard=shard3, table=table3, ald=ald3),
    }

    def ones_bcast(pools, row_ap, nrows, ncols, dst):
        """dst[nrows, ncols] = row broadcast via K=1 matmul."""
        pp = pools["ps"].tile([nrows, ncols], f32, space="PSUM", tag="bc_ps")
        nc.tensor.matmul(out=pp[:], lhsT=ones_row[0:1, 0:nrows], rhs=row_ap,
                         start=True, stop=True)
        nc.vector.tensor_copy(out=dst, in_=pp[:])

    def build_wext(pools, l):
        cfg = LCFG[l]
        heads, F, We = cfg["heads"], cfg["F"], cfg["We"]
        nk = F // P
        wsrc = {1: W1_in, 2: W2_in, 3: W3_in}[l]
        hc = heads * 64
        for k in range(nk):
            nc.sync.dma_start(out=We[:, k, 0:hc], in_=wsrc[k * P:(k + 1) * P, :])
        asr = pools["sb"].tile([P, hc], f32, tag="arep")
        adr = pools["sb"].tile([P, hc], f32, tag="arep")
        arow = pools["sb"].tile([1, hc], f32, tag="arow")
        nc.sync.dma_start(out=arow[:, :], in_=a_in[l, "s"][:, :])
        ones_bcast(pools, arow[:, :], P, hc, asr[:, :])
        arow2 = pools["sb"].tile([1, hc], f32, tag="arow")
        nc.sync.dma_start(out=arow2[:, :], in_=a_in[l, "d"][:, :])
        ones_bcast(pools, arow2[:, :], P, hc, adr[:, :])
        for k in range(nk):
            for (arep, col) in ((asr, hc), (adr, hc + heads)):
                prod = pools["sb"].tile([P, hc], f32, tag="wprod")
                nc.vector.tensor_mul(out=prod[:, :], in0=We[:, k, 0:hc], in1=arep[:, :])
                nc.vector.tensor_reduce(
                    out=We[:, k, col:col + heads],
                    in_=prod[:, :].rearrange("p (h c) -> p h c", h=heads),
                    axis=mybir.AxisListType.X, op=AT.add)
        if l in (1, 2):
            # fold BN scale g' into W columns; B'' = beta - mean*g' + b*g'
            grow = pools["sb"].tile([1, hc], f32, tag="grow")
            t0 = pools["sb"].tile([1, hc], f32, tag="trow")
            nc.sync.dma_start(out=t0[:, :], in_=row_in[f"bn{l}_var"][:, :])
            nc.vector.tensor_scalar_add(grow[:, :], t0[:, :], EPS)
            nc.scalar.activation(out=grow[:, :], in_=grow[:, :], func=ACTF.Sqrt)
            nc.vector.reciprocal(grow[:, :], grow[:, :])
            gam = pools["sb"].tile([1, hc], f32, tag="trow")
            nc.sync.dma_start(out=gam[:, :], in_=row_in[f"bn{l}_gamma"][:, :])
            nc.vector.tensor_mul(out=grow[:, :], in0=grow[:, :], in1=gam[:, :])
            mea = pools["sb"].tile([1, hc], f32, tag="trow")
            nc.sync.dma_start(out=mea[:, :], in_=row_in[f"bn{l}_mean"][:, :])
            bet = pools["sb"].tile([1, hc], f32, tag="trow")
            nc.sync.dma_start(out=bet[:, :], in_=row_in[f"bn{l}_beta"][:, :])
            bia = pools["sb"].tile([1, hc], f32, tag="trow")
            nc.sync.dma_start(out=bia[:, :], in_=row_in[f"b{l}"][:, :])
            brow = pools["sb"].tile([1, hc], f32, tag="brow")
            nc.vector.tensor_mul(out=mea[:, :], in0=mea[:, :], in1=grow[:, :])
            nc.vector.tensor_sub(out=brow[:, :], in0=bet[:, :], in1=mea[:, :])
            nc.vector.tensor_mul(out=bia[:, :], in0=bia[:, :], in1=grow[:, :])
            nc.vector.tensor_add(out=brow[:, :], in0=brow[:, :], in1=bia[:, :])
            ones_bcast(pools, brow[:, :], P, hc, Brep[l][:, :])
            grep = pools["sb"].tile([P, hc], f32, tag="grep")
            ones_bcast(pools, grow[:, :], P, hc, grep[:, :])
            for k in range(nk):
                nc.vector.tensor_mul(out=We[:, k, 0:hc], in0=We[:, k, 0:hc],
                                     in1=grep[:, :])
        else:
            brow = pools["sb"].tile([1, 64], f32, tag="brow")
            nc.sync.dma_start(out=brow[:, :], in_=b3_in[:, :])
            ones_bcast(pools, brow[:, :], P, 64, Brep[3][:, :])

    def dense_chunk(pools, l, i, from_dram=False):
        """Transpose node-chunk i, matmul with Wext, write table-shard rows
        and al_d rows to DRAM."""
        cfg = LCFG[l]
        heads, F, We = cfg["heads"], cfg["F"], cfg["We"]
        hc = heads * 64
        ncols = hc + 2 * heads
        nk = F // P
        nr = min(P, SHN - i * P)
        if from_dram:
            xc = pools["sb"].tile([P, F], f32, tag="xc")
            if nr < P:
                nc.vector.memset(xc[(nr // 32) * 32:P, :], 0.0)
            nc.sync.dma_start(out=xc[0:nr, :], in_=x_in[i * P:i * P + nr, :])
            srcap = lambda k: xc[:, k * P:(k + 1) * P]
        else:
            srcap = lambda k: xnext[:, i, k * P:(k + 1) * P]
        hp = pools["ps"].tile([P, 264], f32, space="PSUM", tag="dense_ps")
        for k in range(nk):
            tp = pools["ps"].tile([P, P], f32, space="PSUM", tag="tr_ps")
            nc.tensor.transpose(out=tp[:], in_=srcap(k), identity=ident[:, :])
            xT = pools["sb"].tile([P, P], f32, tag="xT")
            nc.vector.tensor_copy(out=xT[:], in_=tp[:])
            nc.tensor.matmul(out=hp[0:nr, 0:ncols], lhsT=xT[:, 0:nr],
                             rhs=We[:, k, 0:ncols],
                             start=(k == 0), stop=(k == nk - 1))
        tabt = pools["sb"].tile([P, ROW1], tdt, tag="tabt")
        nc.vector.memset(tabt[0:nr, cfg["alc"] + heads:cfg["row"]], 0.0)
        for hd in range(heads):
            nc.vector.tensor_copy(out=tabt[0:nr, hd * 65:hd * 65 + 64],
                                  in_=hp[0:nr, hd * 64:(hd + 1) * 64])
            nc.vector.memset(tabt[0:nr, hd * 65 + 64:hd * 65 + 65], 1.0)
        nc.vector.tensor_copy(out=tabt[0:nr, cfg["alc"]:cfg["alc"] + heads],
                              in_=hp[0:nr, hc:hc + heads])
        aldt = pools["sb"].tile([P, 4], f32, tag="aldt")
        nc.vector.tensor_copy(out=aldt[0:nr, 0:heads],
                              in_=hp[0:nr, hc + heads:hc + 2 * heads])
        nc.sync.dma_start(out=cfg["shard"][i * P:i * P + nr, 0:cfg["row"]],
                          in_=tabt[0:nr, 0:cfg["row"]])
        nc.sync.dma_start(out=cfg["ald"][i * P:i * P + nr, 0:heads],
                          in_=aldt[0:nr, 0:heads])

    def post_chunk(pools, l, i, psum):
        """num/den divide (+BN shift, relu) for node-chunk i from agg psum."""
        cfg = LCFG[l]
        heads = cfg["heads"]
        pv = psum[:, 0:cfg["hhc"]].rearrange("p (h c) -> p h c", c=65)
        rec = pools["sb"].tile([P, heads], f32, tag="rec")
        nc.vector.reciprocal(rec[:, :], pv[:, :, 64])
        if l in (1, 2):
            xv = xnext[:, i, :].rearrange("p (h c) -> p h c", c=64)
            nc.vector.tensor_mul(
                out=xv, in0=pv[:, :, 0:64],
                in1=rec[:, :, None].to_broadcast([P, heads, 64]))
            nc.vector.tensor_add(out=xnext[:, i, :], in0=xnext[:, i, :],
                                 in1=Brep[l][:, :])
            nc.vector.tensor_relu(out=xnext[:, i, :], in_=xnext[:, i, :])
        else:
            hv = hfext[:, i, 0:64]
            nc.vector.tensor_scalar_mul(hv, psum[:, 0:64], rec[:, 0:1])
            nc.vector.tensor_add(out=hv, in0=hv, in1=Brep[3][:, :])
            nc.vector.tensor_relu(out=hv, in_=hv)

    def edge_phase(pools, l, after_chunk):
        """Gather + softmax + window-matmul aggregation; after node-chunk i
        completes, call after_chunk(i)."""
        import os as _os
        emode = _os.environ.get("KERNEL_EDGE_MODE", "full")
        cfg = LCFG[l]
        heads, row, hhc, alc = cfg["heads"], cfg["row"], cfg["hhc"], cfg["alc"]
        table = cfg["table"]
        psum_cur = [None]
        nch_lim = nch if emode == "full" else (1 if emode == "gather1" else 2)
        for ch in range(nch_lim):
            t0 = ch * KCH
            kk = min(KCH, T - t0)
            St = pools["sb"].tile([P, KCH * W], tdt, tag="St")
            nc.sync.dma_start(out=St[:, 0:kk * W], in_=S_in[ch, :, 0:kk * W])
            sit = pools["sb"].tile([P, KCH * 8], i16, tag="sit")
            nc.sync.dma_start(out=sit[:, 0:kk * 8], in_=si_in[ch, :, 0:kk * 8])
            dit = pools["sb"].tile([P, KCH], i32, tag="dit")
            nc.sync.dma_start(out=dit[:, 0:kk], in_=di_in[ch, :, 0:kk])
            G = pools["gb"].tile([P, KCH, row], tdt, tag="G")
            if emode == "dmas":
                if ch == 0:
                    gd = pools["sb"].tile([P, ROW1], f32, name="gd", tag="gd")
                    nc.vector.memset(gd[:, :], 0.0)
                    nc.vector.tensor_copy(out=gd[:, 0:kk], in_=St[:, 0:kk])
                    nc.sync.dma_start(out=dbg_ext[:, :], in_=gd[:, :])
                continue
            nc.gpsimd.dma_gather(G[:, 0:kk, :], table[:, :], sit[:, 0:kk * 8],
                                 kk * P, kk * P, row,
                                 single_packet=(kk * P * 2 <= 4096))
            ald = pools["sb"].tile([P, KCH, 4], f32, tag="ald")
            if emode in ("gather", "gather1"):
                # validate the big row gather alone; dump G tile 0
                if ch == 0:
                    gd = pools["sb"].tile([P, ROW1], f32, name="gd", tag="gd")
                    nc.vector.memset(gd[:, :], 0.0)
                    nc.vector.tensor_copy(out=gd[:, 0:row], in_=G[:, 0, :])
                    nc.sync.dma_start(out=dbg_ext[:, :], in_=gd[:, :])
                continue
            nc.gpsimd.indirect_dma_start(
                out=ald[:, 0:kk, 0:heads], out_offset=None,
                in_=cfg["ald"][:, :],
                in_offset=bass.IndirectOffsetOnAxis(ap=dit[:, 0:kk], axis=0))
            if emode == "indirect":
                if ch == 0:
                    gd = pools["sb"].tile([P, ROW1], f32, name="gd", tag="gd")
                    nc.vector.memset(gd[:, :], 0.0)
                    nc.vector.tensor_copy(out=gd[:, 0:kk * heads],
                                          in_=ald[:, 0:kk, 0:heads]
                                          .rearrange("p k h -> p (k h)"))
                    nc.sync.dma_start(out=dbg_ext[:, :], in_=gd[:, :])
                continue
            ee = pools["sb"].tile([P, KCH * heads], f32, tag="ee")
            eek = ee[:, 0:kk * heads]
            nc.vector.tensor_add(
                out=eek.rearrange("p (k h) -> p k h", h=heads),
                in0=G[:, 0:kk, alc:alc + heads],
                in1=ald[:, 0:kk, 0:heads])
            t2 = pools["sb"].tile([P, KCH * heads], f32, tag="eet")
            nc.vector.tensor_scalar_mul(t2[:, 0:kk * heads], eek, SLOPE)
            nc.vector.tensor_max(out=eek, in0=eek, in1=t2[:, 0:kk * heads])
            eex = pools["sb"].tile([P, KCH * heads], tdt, tag="eex")
            nc.scalar.activation(out=eex[:, 0:kk * heads], in_=eek, func=ACTF.Exp)
            hh = pools["gb"].tile([P, KCH, hhc], tdt, tag="hh")
            nc.vector.tensor_mul(
                out=hh[:, 0:kk, :].rearrange("p k (h c) -> p k h c", h=heads),
                in0=G[:, 0:kk, 0:hhc].rearrange("p k (h c) -> p k h c", h=heads),
                in1=eex[:, 0:kk * heads].rearrange("p (k h) -> p k h", h=heads)
                    [:, :, :, None].to_broadcast([P, kk, heads, 65]))
            if emode == "dve":
                if ch == 0:
                    gd = pools["sb"].tile([P, ROW1], f32, name="gd", tag="gd")
                    nc.vector.memset(gd[:, :], 0.0)
                    nc.vector.tensor_copy(out=gd[:, 0:hhc], in_=hh[:, 0, :])
                    nc.sync.dma_start(out=dbg_ext[:, :], in_=gd[:, :])
                continue
            for j in range(kk):
                t = t0 + j
                w = win_of_tile[t]
                slot = w % 4
                i = w // 4
                if slot == 0 and win_start[w] == t:
                    psum_cur[0] = pools["ps"].tile([P, 512], f32, space="PSUM",
                                                   tag="agg_ps", name="agg_ps")
                ps = psum_cur[0]
                nc.tensor.matmul(
                    out=ps[32 * slot:32 * (slot + 1), 0:hhc],
                    lhsT=St[:, j * W:(j + 1) * W], rhs=hh[:, j, :],
                    start=(win_start[w] == t), stop=(win_end[w] == t),
                    tile_position=(0, 32 * slot))
                if slot == 3 and win_end[w] == t:
                    post_chunk(pools, l, i, ps)
                    after_chunk(i)

    def mk_pools(tc, stk):
        return dict(
            sb=stk.enter_context(tc.tile_pool(name="sb", bufs=3)),
            gb=stk.enter_context(tc.tile_pool(name="gb", bufs=2)),
            ps=stk.enter_context(tc.tile_pool(name="ps", bufs=2, space="PSUM")),
        )

    def collective(kind, op, in_dram, out_dram, sem):
        nc.gpsimd.collective_compute(
            kind, op, ins=[in_dram[:, :].opt()], outs=[out_dram[:, :].opt()],
            replica_groups=[list(range(NC))]).then_inc(sem, 1)
        nc.gpsimd.wait_ge(sem, 1)
        # other engines must not enter the next TileContext before the
        # collective's output is visible
        nc.all_engine_barrier()

    # ---- TC0: constants + Wext(all layers) + dense layer 1 ----
    with tile.TileContext(nc) as tc, ExitStack() as stk:
        pools = mk_pools(tc, stk)
        make_identity(nc, ident[:, :])
        nc.vector.memset(ones_row[:, :], 1.0)
        nc.vector.memset(hfext[:, :, 64:65], 1.0)
        nc.sync.dma_start(out=lin1e[0:64, :], in_=l1w_in[:, :])
        nc.sync.dma_start(out=lin1e[64:65, :], in_=l1b_in[:, :])
        nc.sync.dma_start(out=lin2e[0:32, :], in_=l2w_in[:, :])
        nc.sync.dma_start(out=lin2e[32:33, :], in_=l2b_in[:, :])
        for l in (1, 2, 3):
            build_wext(pools, l)
        for i in range(NCHK):
            dense_chunk(pools, 1, i, from_dram=True)
    def dump_dram(src_dram, rows, cols):
        with tile.TileContext(nc) as tc, ExitStack() as stk:
            pools = mk_pools(tc, stk)
            d = pools["sb"].tile([P, ROW1], f32, name="d")
            nc.vector.memset(d[:, :], 0.0)
            dd = pools["sb"].tile([P, ROW1], tdt, name="dd")
            nc.sync.dma_start(out=dd[0:rows, 0:cols], in_=src_dram)
            nc.vector.tensor_copy(out=d[0:rows, 0:cols], in_=dd[0:rows, 0:cols])
            nc.sync.dma_start(out=dbg_ext[:, :], in_=d[:, :])

    def dump_sbuf(src_ap, rows, cols):
        import os as _os
        if _os.environ.get("KERNEL_EDGE_MODE", "full") != "full":
            return
        with tile.TileContext(nc) as tc, ExitStack() as stk:
            pools = mk_pools(tc, stk)
            d = pools["sb"].tile([P, ROW1], f32, name="d")
            nc.vector.memset(d[:, :], 0.0)
            nc.vector.tensor_copy(out=d[0:rows, 0:cols], in_=src_ap)
            nc.sync.dma_start(out=dbg_ext[:, :], in_=d[:, :])

    if phases == 0:
        dump_dram(shard12[0:P, 0:ROW1], P, ROW1)
        nc.compile()
        es.close()
        return nc
    collective("AllGather", AT.bypass, shard12, table12, cc_sems[0])
    if phases == 1:
        dump_dram(table12[3 * SHN:3 * SHN + P, 0:ROW1], P, ROW1)
        nc.compile()
        es.close()
        return nc

    # ---- TC1: edges layer 1 + dense layer 2 ----
    with tile.TileContext(nc) as tc, ExitStack() as stk:
        pools = mk_pools(tc, stk)
        edge_phase(pools, 1, lambda i: dense_chunk(pools, 2, i))
    if phases == 2:
        dump_sbuf(xnext[:, 0, 0:256], P, 256)
        nc.compile()
        es.close()
        return nc
    collective("AllGather", AT.bypass, shard12, table12, cc_sems[1])

    # ---- TC2: edges layer 2 + dense layer 3 ----
    with tile.TileContext(nc) as tc, ExitStack() as stk:
        pools = mk_pools(tc, stk)
        edge_phase(pools, 2, lambda i: dense_chunk(pools, 3, i))
    if phases == 3:
        dump_sbuf(xnext[:, 0, 0:256], P, 256)
        nc.compile()
        es.close()
        return nc
    collective("AllGather", AT.bypass, shard3, table3, cc_sems[2])

    # ---- TC3: edges layer 3 + readout partials ----
    with tile.TileContext(nc) as tc, ExitStack() as stk:
        pools = mk_pools(tc, stk)
        ro_ps = pools["ps"].tile([NGR, HID + 1], f32, space="PSUM", tag="ro_ps")

        def ro_chunk(i):
            Sbt = pools["sb"].tile([P, NGR], f32, tag="Sbt")
            nc.sync.dma_start(out=Sbt[:, :], in_=Sb_in[i, :, :])
            nc.tensor.matmul(out=ro_ps[:], lhsT=Sbt[:, :], rhs=hfext[:, i, :],
                             start=(i == 0), stop=(i == NCHK - 1))
            if i == NCHK - 1:
                ro_sb = pools["sb"].tile([NGR, HID + 1], f32, tag="ro_sb")
                nc.vector.tensor_copy(out=ro_sb[:], in_=ro_ps[:])
                nc.sync.dma_start(out=ro_in[:, :], in_=ro_sb[:])

        edge_phase(pools, 3, ro_chunk)
    if phases == 4:
        dump_dram(ro_in[:, :], NGR, HID + 1)
        nc.compile()
        es.close()
        return nc
    collective("AllReduce", AT.add, ro_in, ro_out, cc_sems[3])

    # ---- TC4: final MLP ----
    with tile.TileContext(nc) as tc, ExitStack() as stk:
        pools = mk_pools(tc, stk)
        ro = pools["sb"].tile([NGR, HID + 1], f32)
        nc.sync.dma_start(out=ro[:], in_=ro_out[:, :])
        cnt = pools["sb"].tile([NGR, 1], f32)
        nc.vector.tensor_scalar_max(cnt[:], ro[:, 64:65], 1.0)
        rec = pools["sb"].tile([NGR, 1], f32)
        nc.vector.reciprocal(rec[:], cnt[:])
        gext = pools["sb"].tile([NGR, HID + 1], f32)
        nc.vector.tensor_scalar_mul(gext[:, 0:64], ro[:, 0:64], rec[:, 0:1])
        nc.vector.memset(gext[:, 64:65], 1.0)
        gT_ps = pools["ps"].tile([HID + 1, NGR], f32, space="PSUM")
        nc.tensor.transpose(out=gT_ps[:], in_=gext[:], identity=ident[0:NGR, 0:NGR])
        gT = pools["sb"].tile([HID + 1, NGR], f32)
        nc.vector.tensor_copy(out=gT[:], in_=gT_ps[:])
        o1_ps = pools["ps"].tile([NGR, 32], f32, space="PSUM")
        nc.tensor.matmul(out=o1_ps[:], lhsT=gT[:], rhs=lin1e[:, :],
                         start=True, stop=True)
        o1e = pools["sb"].tile([NGR, 33], f32)
        nc.vector.tensor_relu(out=o1e[:, 0:32], in_=o1_ps[:])
        nc.vector.memset(o1e[:, 32:33], 1.0)
        o1T_ps = pools["ps"].tile([33, NGR], f32, space="PSUM")
        nc.tensor.transpose(out=o1T_ps[:], in_=o1e[:], identity=ident[0:NGR, 0:NGR])
        o1T = pools["sb"].tile([33, NGR], f32)
        nc.vector.tensor_copy(out=o1T[:], in_=o1T_ps[:])
        o2_ps = pools["ps"].tile([NGR, NCLS], f32, space="PSUM")
        nc.tensor.matmul(out=o2_ps[:], lhsT=o1T[:], rhs=lin2e[:, :],
                         start=True, stop=True)
        o2 = pools["sb"].tile([NGR, NCLS], f32)
        nc.vector.tensor_copy(out=o2[:], in_=o2_ps[:])
        nc.sync.dma_start(out=out_ext[:, :], in_=o2[:])

    nc.compile()
    es.close()
    return nc


def _bf16():
    from ml_dtypes import bfloat16
    return bfloat16


_CACHE = {}


def _get_built(ei, batch):
    import os
    if "k" not in _CACHE:
        core_arrays, win_of_tile, tiles_per_win, nch = _preprocess(ei)
        SbL = _batch_S(batch)
        nc = _build(win_of_tile, nch, phases=int(os.environ.get("KERNEL_PHASES", "5")))
        _CACHE["k"] = (nc, core_arrays, SbL)
    return _CACHE["k"]


def _in_maps(inputs, core_arrays, SbL):
    x = np.ascontiguousarray(np.asarray(inputs["x"], np.float32))
    maps = []
    for c in range(NC):
        m = dict(
            x_sh=x[c * SHN:(c + 1) * SHN],
            W1=np.asarray(inputs["W1"], np.float32),
            W2=np.asarray(inputs["W2"], np.float32),
            W3=np.asarray(inputs["W3"], np.float32),
            a1s=np.asarray(inputs["a1_src"], np.float32).reshape(1, 256),
            a1d=np.asarray(inputs["a1_dst"], np.float32).reshape(1, 256),
            a2s=np.asarray(inputs["a2_src"], np.float32).reshape(1, 256),
            a2d=np.asarray(inputs["a2_dst"], np.float32).reshape(1, 256),
            a3s=np.asarray(inputs["a3_src"], np.float32).reshape(1, 64),
            a3d=np.asarray(inputs["a3_dst"], np.float32).reshape(1, 64),
            b3=np.asarray(inputs["b3"], np.float32).reshape(1, 64),
            lin1_w=np.asarray(inputs["lin1_w"], np.float32),
            lin1_b=np.asarray(inputs["lin1_b"], np.float32).reshape(1, 32),
            lin2_w=np.asarray(inputs["lin2_w"], np.float32),
            lin2_b=np.asarray(inputs["lin2_b"], np.float32).reshape(1, 5),
            si=core_arrays[c]["si"],
            di=core_arrays[c]["di"],
            S=(core_arrays[c]["S"].astype(_bf16()) if BF16_TABLE
               else core_arrays[c]["S"]),
            Sb=SbL[c],
        )
        for nm in ("b1", "b2", "bn1_gamma", "bn1_beta", "bn1_mean", "bn1_var",
                   "bn2_gamma", "bn2_beta", "bn2_mean", "bn2_var"):
            m[nm] = np.asarray(inputs[nm], np.float32).reshape(1, 256)
        maps.append(m)
    return maps


def kernel(**inputs):
    from concourse.bass_utils import run_bass_kernel_spmd
    ei = np.asarray(inputs["edge_index"])
    batch = np.asarray(inputs["batch"])
    nc, core_arrays, SbL = _get_built(ei, batch)
    maps = _in_maps(inputs, core_arrays, SbL)
    res = run_bass_kernel_spmd(nc, maps, core_ids=list(range(NC)))
    return np.asarray(res.results[0]["out"], np.float32)


# revision 21
# speedup vs baseline: 1.0210x; 1.0210x over previous
"""Distributed 3-layer GAT (BioGNN) for 8 TRN2 NeuronCores.

Sharding: nodes partitioned into 8 contiguous shards of 2500; each core owns
the edges whose dst lies in its shard (dst-sorted, grouped into 32-dst
windows, padded to 128-edge tiles; per-window tile counts padded to the max
across cores so all cores run one SPMD program). Per layer: a local dense
matmul produces a node-feature table shard ([h|1|...|al_s] rows) which is
AllGathered to every core; the edge phase dma_gathers source rows per edge,
computes softmax weights on DVE/ACT, and aggregates per dst-window with
selection-matrix matmuls accumulating in PSUM. Readout is a batch-one-hot
matmul, AllReduced, finished with a small on-chip MLP.

Host-side work is integer index preprocessing only; all float math runs on
the NeuronCores.
"""
import sys
sys.path.insert(0, "/opt/trn_rl_repo")
import numpy as np

NC = 8
N = 20000
SHN = N // NC            # 2500
FIN = 512
HID = 64
NGR = 64
NCLS = 5
P = 128
W = 32                   # dst window
KCH = 8                  # tiles per gather chunk (1024-idx gathers: larger crashes on Shared tables)
ROW1 = 320               # layer1/2 table row elems (set after BF16_TABLE below)
ROW3 = 128               # layer3 table row floats
SLOPE = 0.2
EPS = 1e-5
BF16_TABLE = False       # table/S/gather/hh in bf16 (2x gather+exchange, ~2.4e-4 rel err)
ROW1 = 384 if BF16_TABLE else 320   # layer1/2 table row elems (row bytes % 256 == 0)
NCHK = (SHN + P - 1) // P                    # 20 node chunks per core
NWIN = ((SHN + W - 1) // W + 3) // 4 * 4     # 80 windows


def _preprocess(edge_index):
    """Index-only host work. Uniform program: per-window tile counts padded to
    the max across cores. Returns per-core arrays + global tile->window map."""
    src = np.concatenate([edge_index[0].astype(np.int64), np.arange(N, dtype=np.int64)])
    dst = np.concatenate([edge_index[1].astype(np.int64), np.arange(N, dtype=np.int64)])
    per_core = []
    for c in range(NC):
        lo = c * SHN
        m = (dst >= lo) & (dst < lo + SHN)
        es, ed = src[m], dst[m] - lo
        order = np.argsort(ed, kind="stable")
        es, ed = es[order], ed[order]
        wins = []
        for w in range(NWIN):
            wm = (ed >= w * W) & (ed < (w + 1) * W)
            ws, wd = es[wm], (ed[wm] - w * W)
            # fake dst slots (beyond SHN) get one S=1 edge so denom != 0
            fake_lo = max(0, min(W, SHN - w * W))
            nfk = W - fake_lo
            ws = np.concatenate([ws, np.zeros(nfk, np.int64)])
            wd = np.concatenate([wd, np.arange(fake_lo, W, dtype=np.int64)])
            val = np.ones(len(ws), np.float32)
            wins.append((ws, wd, val))
        per_core.append(wins)

    tiles_per_win = []
    for w in range(NWIN):
        mx = max(len(per_core[c][w][0]) for c in range(NC))
        tiles_per_win.append(max(1, (mx + P - 1) // P))
    T = sum(tiles_per_win)
    win_of_tile = []
    for w, nt in enumerate(tiles_per_win):
        win_of_tile += [w] * nt

    nch = (T + KCH - 1) // KCH
    Tpad = nch * KCH
    core_arrays = []
    for c in range(NC):
        srcs = np.zeros((Tpad, P), np.int64)
        dsts = np.zeros((Tpad, P), np.int64)
        Sarr = np.zeros((Tpad, P, W), np.float32)
        t0 = 0
        for w, nt in enumerate(tiles_per_win):
            ws, wd, val = per_core[c][w]
            ne = len(ws)
            pad = nt * P - ne
            ws = np.concatenate([ws, np.zeros(pad, np.int64)])
            wd = np.concatenate([wd, np.zeros(pad, np.int64)])
            val = np.concatenate([val, np.zeros(pad, np.float32)])
            for t in range(nt):
                sl = slice(t * P, (t + 1) * P)
                srcs[t0 + t] = ws[sl]
                dloc = w * W + wd[sl]
                dsts[t0 + t] = np.where(dloc < SHN, dloc, 0)
                Sarr[t0 + t][np.arange(P), wd[sl]] = val[sl]
            t0 += nt
        # dma_gather idx: per chunk [128, KCH*8] int16; idx position i=j*128+p
        # -> wrapped [i%16, i//16], replicated x8 over the 8 partition groups
        flat = srcs.reshape(nch, KCH * P).astype(np.int16)
        si = np.zeros((nch, P, KCH * 8), np.int16)
        for ch in range(nch):
            w16 = flat[ch].reshape(KCH * 8, 16).T           # [16, KCH*8]
            si[ch] = np.tile(w16, (8, 1))
        # indirect al_d idx: per chunk [128, KCH] int32; [p, j] = local dst of
        # (tile j, partition p); consumed partition-major (p*KCH + j)
        di = dsts.reshape(nch, KCH, P).transpose(0, 2, 1).astype(np.int32)
        Sc = Sarr.reshape(nch, KCH, P, W).transpose(0, 2, 1, 3).reshape(nch, P, KCH * W)
        core_arrays.append(dict(si=np.ascontiguousarray(si),
                                di=np.ascontiguousarray(di),
                                S=np.ascontiguousarray(Sc)))
    return core_arrays, win_of_tile, tiles_per_win, nch


def _batch_S(batch):
    out = []
    for c in range(NC):
        lo = c * SHN
        Sb = np.zeros((NCHK, P, NGR), np.float32)
        for i in range(NCHK):
            b = batch[lo + i * P: min(lo + (i + 1) * P, lo + SHN)]
            Sb[i][np.arange(len(b)), b] = 1.0
        out.append(Sb)
    return out


def _build(win_of_tile, nch, phases=5):
    import concourse.bass as bass
    import concourse.bacc as bacc
    import concourse.mybir as mybir
    import concourse.tile as tile
    from concourse.masks import make_identity
    from contextlib import ExitStack

    f32 = mybir.dt.float32
    tdt = mybir.dt.bfloat16 if BF16_TABLE else f32
    i16 = mybir.dt.int16
    i32 = mybir.dt.int32
    AT = mybir.AluOpType
    ACTF = mybir.ActivationFunctionType
    T = len(win_of_tile)
    win_start = {}
    win_end = {}
    for t, w in enumerate(win_of_tile):
        win_start.setdefault(w, t)
        win_end[w] = t

    nc = bacc.Bacc(num_devices=NC)
    dp = nc.declare_dram_parameter
    x_in = dp("x_sh", [SHN, FIN], f32, isOutput=False)
    W1_in = dp("W1", [FIN, 256], f32, isOutput=False)
    W2_in = dp("W2", [256, 256], f32, isOutput=False)
    W3_in = dp("W3", [256, 64], f32, isOutput=False)
    a_in = {}
    for l, h in ((1, 4), (2, 4), (3, 1)):
        a_in[l, "s"] = dp(f"a{l}s", [1, h * 64], f32, isOutput=False)
        a_in[l, "d"] = dp(f"a{l}d", [1, h * 64], f32, isOutput=False)
    row_in = {}
    for nm in ("b1", "b2", "bn1_gamma", "bn1_beta", "bn1_mean", "bn1_var",
               "bn2_gamma", "bn2_beta", "bn2_mean", "bn2_var"):
        row_in[nm] = dp(nm, [1, 256], f32, isOutput=False)
    b3_in = dp("b3", [1, 64], f32, isOutput=False)
    l1w_in = dp("lin1_w", [64, 32], f32, isOutput=False)
    l1b_in = dp("lin1_b", [1, 32], f32, isOutput=False)
    l2w_in = dp("lin2_w", [32, 5], f32, isOutput=False)
    l2b_in = dp("lin2_b", [1, 5], f32, isOutput=False)
    si_in = dp("si", [nch, P, KCH * 8], i16, isOutput=False)
    di_in = dp("di", [nch, P, KCH], i32, isOutput=False)
    S_in = dp("S", [nch, P, KCH * W], tdt, isOutput=False)
    Sb_in = dp("Sb", [NCHK, P, NGR], f32, isOutput=False)
    out_ext = dp("out", [NGR, NCLS], f32, isOutput=True)
    dbg_ext = dp("dbg", [P, ROW1], f32, isOutput=True)

    shard12 = nc.dram_tensor("shard12", [SHN, ROW1], tdt)
    table12 = nc.dram_tensor("table12", [N, ROW1], tdt, addr_space="Shared")
    shard3 = nc.dram_tensor("shard3", [SHN, ROW3], tdt)
    table3 = nc.dram_tensor("table3", [N, ROW3], tdt, addr_space="Shared")
    ald12 = nc.dram_tensor("ald12", [SHN, 4], f32)
    ald3 = nc.dram_tensor("ald3", [SHN, 1], f32)
    ro_in = nc.dram_tensor("ro_in", [NGR, HID + 1], f32)
    ro_out = nc.dram_tensor("ro_out", [NGR, HID + 1], f32, addr_space="Shared")

    es = ExitStack()
    sbt = lambda name, shape, dt=f32: es.enter_context(nc.sbuf_tensor(name, shape, dt))
    ident = sbt("ident", [P, P])
    ones_row = sbt("ones_row", [1, P])
    W1e = sbt("W1e", [P, 4, 264])
    W2e = sbt("W2e", [P, 2, 264])
    W3e = sbt("W3e", [P, 2, 66])
    Brep = {1: sbt("B1rep", [P, 256]), 2: sbt("B2rep", [P, 256]),
            3: sbt("B3rep", [P, 64])}
    xnext = sbt("xnext", [P, NCHK, 256])
    hfext = sbt("hfext", [P, NCHK, HID + 1])
    lin1e = sbt("lin1e", [65, 32])
    lin2e = sbt("lin2e", [33, 5])
    cc_sems = [es.enter_context(nc.semaphore(f"cc{i}")) for i in range(4)]

    LCFG = {
        1: dict(heads=4, F=FIN, We=W1e, row=ROW1, hhc=260, alc=260,
                shard=shard12, table=table12, ald=ald12),
        2: dict(heads=4, F=256, We=W2e, row=ROW1, hhc=260, alc=260,
                shard=shard12, table=table12, ald=ald12),
        3: dict(heads=1, F=256, We=W3e, row=ROW3, hhc=65, alc=65,
                shard=shard3, table=table3, ald=ald3),
    }

    def ones_bcast(pools, row_ap, nrows, ncols, dst):
        """dst[nrows, ncols] = row broadcast via K=1 matmul."""
        pp = pools["ps"].tile([nrows, ncols], f32, space="PSUM", tag="bc_ps")
        nc.tensor.matmul(out=pp[:], lhsT=ones_row[0:1, 0:nrows], rhs=row_ap,
                         start=True, stop=True)
        nc.vector.tensor_copy(out=dst, in_=pp[:])

    def build_wext(pools, l):
        cfg = LCFG[l]
        heads, F, We = cfg["heads"], cfg["F"], cfg["We"]
        nk = F // P
        wsrc = {1: W1_in, 2: W2_in, 3: W3_in}[l]
        hc = heads * 64
        for k in range(nk):
            nc.sync.dma_start(out=We[:, k, 0:hc], in_=wsrc[k * P:(k + 1) * P, :])
        asr = pools["sb"].tile([P, hc], f32, tag="arep")
        adr = pools["sb"].tile([P, hc], f32, tag="arep")
        arow = pools["sb"].tile([1, hc], f32, tag="arow")
        nc.sync.dma_start(out=arow[:, :], in_=a_in[l, "s"][:, :])
        ones_bcast(pools, arow[:, :], P, hc, asr[:, :])
        arow2 = pools["sb"].tile([1, hc], f32, tag="arow")
        nc.sync.dma_start(out=arow2[:, :], in_=a_in[l, "d"][:, :])
        ones_bcast(pools, arow2[:, :], P, hc, adr[:, :])
        for k in range(nk):
            for (arep, col) in ((asr, hc), (adr, hc + heads)):
                prod = pools["sb"].tile([P, hc], f32, tag="wprod")
                nc.vector.tensor_mul(out=prod[:, :], in0=We[:, k, 0:hc], in1=arep[:, :])
                nc.vector.tensor_reduce(
                    out=We[:, k, col:col + heads],
                    in_=prod[:, :].rearrange("p (h c) -> p h c", h=heads),
                    axis=mybir.AxisListType.X, op=AT.add)
        if l in (1, 2):
            # fold BN scale g' into W columns; B'' = beta - mean*g' + b*g'
            grow = pools["sb"].tile([1, hc], f32, tag="grow")
            t0 = pools["sb"].tile([1, hc], f32, tag="trow")
            nc.sync.dma_start(out=t0[:, :], in_=row_in[f"bn{l}_var"][:, :])
            nc.vector.tensor_scalar_add(grow[:, :], t0[:, :], EPS)
            nc.scalar.activation(out=grow[:, :], in_=grow[:, :], func=ACTF.Sqrt)
            nc.vector.reciprocal(grow[:, :], grow[:, :])
            gam = pools["sb"].tile([1, hc], f32, tag="trow")
            nc.sync.dma_start(out=gam[:, :], in_=row_in[f"bn{l}_gamma"][:, :])
            nc.vector.tensor_mul(out=grow[:, :], in0=grow[:, :], in1=gam[:, :])
            mea = pools["sb"].tile([1, hc], f32, tag="trow")
            nc.sync.dma_start(out=mea[:, :], in_=row_in[f"bn{l}_mean"][:, :])
            bet = pools["sb"].tile([1, hc], f32, tag="trow")
            nc.sync.dma_start(out=bet[:, :], in_=row_in[f"bn{l}_beta"][:, :])
            bia = pools["sb"].tile([1, hc], f32, tag="trow")
            nc.sync.dma_start(out=bia[:, :], in_=row_in[f"b{l}"][:, :])
            brow = pools["sb"].tile([1, hc], f32, tag="brow")
            nc.vector.tensor_mul(out=mea[:, :], in0=mea[:, :], in1=grow[:, :])
            nc.vector.tensor_sub(out=brow[:, :], in0=bet[:, :], in1=mea[:, :])
            nc.vector.tensor_mul(out=bia[:, :], in0=bia[:, :], in1=grow[:, :])
            nc.vector.tensor_add(out=brow[:, :], in0=brow[:, :], in1=bia[:, :])
            ones_bcast(pools, brow[:, :], P, hc, Brep[l][:, :])
            grep = pools["sb"].tile([P, hc], f32, tag="grep")
            ones_bcast(pools, grow[:, :], P, hc, grep[:, :])
            for k in range(nk):
                nc.vector.tensor_mul(out=We[:, k, 0:hc], in0=We[:, k, 0:hc],
                                     in1=grep[:, :])
        else:
            brow = pools["sb"].tile([1, 64], f32, tag="brow")
            nc.sync.dma_start(out=brow[:, :], in_=b3_in[:, :])
            ones_bcast(pools, brow[:, :], P, 64, Brep[3][:, :])

    def dense_chunk(pools, l, i, from_dram=False):
        """Transpose node-chunk i, matmul with Wext, write table-shard rows
        and al_d rows to DRAM."""
        cfg = LCFG[l]
        heads, F, We = cfg["heads"], cfg["F"], cfg["We"]
        hc = heads * 64
        ncols = hc + 2 * heads
        nk = F // P
        nr = min(P, SHN - i * P)
        if from_dram:
            xc = pools["sb"].tile([P, F], f32, tag="xc")
            if nr < P:
                nc.vector.memset(xc[(nr // 32) * 32:P, :], 0.0)
            nc.sync.dma_start(out=xc[0:nr, :], in_=x_in[i * P:i * P + nr, :])
            srcap = lambda k: xc[:, k * P:(k + 1) * P]
        else:
            srcap = lambda k: xnext[:, i, k * P:(k + 1) * P]
        hp = pools["ps"].tile([P, 264], f32, space="PSUM", tag="dense_ps")
        for k in range(nk):
            tp = pools["ps"].tile([P, P], f32, space="PSUM", tag="tr_ps")
            nc.tensor.transpose(out=tp[:], in_=srcap(k), identity=ident[:, :])
            xT = pools["sb"].tile([P, P], f32, tag="xT")
            nc.vector.tensor_copy(out=xT[:], in_=tp[:])
            nc.tensor.matmul(out=hp[0:nr, 0:ncols], lhsT=xT[:, 0:nr],
                             rhs=We[:, k, 0:ncols],
                             start=(k == 0), stop=(k == nk - 1))
        tabt = pools["sb"].tile([P, ROW1], tdt, tag="tabt")
        nc.vector.memset(tabt[0:nr, cfg["alc"] + heads:cfg["row"]], 0.0)
        for hd in range(heads):
            nc.vector.tensor_copy(out=tabt[0:nr, hd * 65:hd * 65 + 64],
                                  in_=hp[0:nr, hd * 64:(hd + 1) * 64])
            nc.vector.memset(tabt[0:nr, hd * 65 + 64:hd * 65 + 65], 1.0)
        nc.vector.tensor_copy(out=tabt[0:nr, cfg["alc"]:cfg["alc"] + heads],
                              in_=hp[0:nr, hc:hc + heads])
        aldt = pools["sb"].tile([P, 4], f32, tag="aldt")
        nc.vector.tensor_copy(out=aldt[0:nr, 0:heads],
                              in_=hp[0:nr, hc + heads:hc + 2 * heads])
        nc.sync.dma_start(out=cfg["shard"][i * P:i * P + nr, 0:cfg["row"]],
                          in_=tabt[0:nr, 0:cfg["row"]])
        nc.sync.dma_start(out=cfg["ald"][i * P:i * P + nr, 0:heads],
                          in_=aldt[0:nr, 0:heads])

    def post_chunk(pools, l, i, psum):
        """num/den divide (+BN shift, relu) for node-chunk i from agg psum."""
        cfg = LCFG[l]
        heads = cfg["heads"]
        pv = psum[:, 0:cfg["hhc"]].rearrange("p (h c) -> p h c", c=65)
        rec = pools["sb"].tile([P, heads], f32, tag="rec")
        nc.vector.reciprocal(rec[:, :], pv[:, :, 64])
        if l in (1, 2):
            xv = xnext[:, i, :].rearrange("p (h c) -> p h c", c=64)
            nc.vector.tensor_mul(
                out=xv, in0=pv[:, :, 0:64],
                in1=rec[:, :, None].to_broadcast([P, heads, 64]))
            nc.vector.tensor_add(out=xnext[:, i, :], in0=xnext[:, i, :],
                                 in1=Brep[l][:, :])
            nc.vector.tensor_relu(out=xnext[:, i, :], in_=xnext[:, i, :])
        else:
            hv = hfext[:, i, 0:64]
            nc.vector.tensor_scalar_mul(hv, psum[:, 0:64], rec[:, 0:1])
            nc.vector.tensor_add(out=hv, in0=hv, in1=Brep[3][:, :])
            nc.vector.tensor_relu(out=hv, in_=hv)

    def edge_phase(pools, l, after_chunk):
        """Gather + softmax + window-matmul aggregation; after node-chunk i
        completes, call after_chunk(i)."""
        import os as _os
        emode = _os.environ.get("KERNEL_EDGE_MODE", "full")
        cfg = LCFG[l]
        heads, row, hhc, alc = cfg["heads"], cfg["row"], cfg["hhc"], cfg["alc"]
        table = cfg["table"]
        psum_cur = [None]
        nch_lim = nch if emode == "full" else (1 if emode == "gather1" else 2)
        for ch in range(nch_lim):
            t0 = ch * KCH
            kk = min(KCH, T - t0)
            St = pools["sb"].tile([P, KCH * W], tdt, tag="St")
            nc.sync.dma_start(out=St[:, 0:kk * W], in_=S_in[ch, :, 0:kk * W])
            sit = pools["sb"].tile([P, KCH * 8], i16, tag="sit")
            nc.sync.dma_start(out=sit[:, 0:kk * 8], in_=si_in[ch, :, 0:kk * 8])
            dit = pools["sb"].tile([P, KCH], i32, tag="dit")
            nc.sync.dma_start(out=dit[:, 0:kk], in_=di_in[ch, :, 0:kk])
            G = pools["gb"].tile([P, KCH, row], tdt, tag="G")
            if emode == "dmas":
                if ch == 0:
                    gd = pools["sb"].tile([P, ROW1], f32, name="gd", tag="gd")
                    nc.vector.memset(gd[:, :], 0.0)
                    nc.vector.tensor_copy(out=gd[:, 0:kk], in_=St[:, 0:kk])
                    nc.sync.dma_start(out=dbg_ext[:, :], in_=gd[:, :])
                continue
            nc.gpsimd.dma_gather(G[:, 0:kk, :], table[:, :], sit[:, 0:kk * 8],
                                 kk * P, kk * P, row,
                                 single_packet=(kk * P * 2 <= 4096))
            ald = pools["sb"].tile([P, KCH, 4], f32, tag="ald")
            if emode in ("gather", "gather1"):
                # validate the big row gather alone; dump G tile 0
                if ch == 0:
                    gd = pools["sb"].tile([P, ROW1], f32, name="gd", tag="gd")
                    nc.vector.memset(gd[:, :], 0.0)
                    nc.vector.tensor_copy(out=gd[:, 0:row], in_=G[:, 0, :])
                    nc.sync.dma_start(out=dbg_ext[:, :], in_=gd[:, :])
                continue
            nc.gpsimd.indirect_dma_start(
                out=ald[:, 0:kk, 0:heads], out_offset=None,
                in_=cfg["ald"][:, :],
                in_offset=bass.IndirectOffsetOnAxis(ap=dit[:, 0:kk], axis=0))
            if emode == "indirect":
                if ch == 0:
                    gd = pools["sb"].tile([P, ROW1], f32, name="gd", tag="gd")
                    nc.vector.memset(gd[:, :], 0.0)
                    nc.vector.tensor_copy(out=gd[:, 0:kk * heads],
                                          in_=ald[:, 0:kk, 0:heads]
                                          .rearrange("p k h -> p (k h)"))
                    nc.sync.dma_start(out=dbg_ext[:, :], in_=gd[:, :])
                continue
            ee = pools["sb"].tile([P, KCH * heads], f32, tag="ee")
            eek = ee[:, 0:kk * heads]
            nc.vector.tensor_add(
                out=eek.rearrange("p (k h) -> p k h", h=heads),
                in0=G[:, 0:kk, alc:alc + heads],
                in1=ald[:, 0:kk, 0:heads])
            t2 = pools["sb"].tile([P, KCH * heads], f32, tag="eet")
            nc.vector.tensor_scalar_mul(t2[:, 0:kk * heads], eek, SLOPE)
            nc.vector.tensor_max(out=eek, in0=eek, in1=t2[:, 0:kk * heads])
            eex = pools["sb"].tile([P, KCH * heads], tdt, tag="eex")
            nc.scalar.activation(out=eex[:, 0:kk * heads], in_=eek, func=ACTF.Exp)
            hh = pools["gb"].tile([P, KCH, hhc], tdt, tag="hh")
            nc.vector.tensor_mul(
                out=hh[:, 0:kk, :].rearrange("p k (h c) -> p k h c", h=heads),
                in0=G[:, 0:kk, 0:hhc].rearrange("p k (h c) -> p k h c", h=heads),
                in1=eex[:, 0:kk * heads].rearrange("p (k h) -> p k h", h=heads)
                    [:, :, :, None].to_broadcast([P, kk, heads, 65]))
            if emode == "dve":
                if ch == 0:
                    gd = pools["sb"].tile([P, ROW1], f32, name="gd", tag="gd")
                    nc.vector.memset(gd[:, :], 0.0)
                    nc.vector.tensor_copy(out=gd[:, 0:hhc], in_=hh[:, 0, :])
                    nc.sync.dma_start(out=dbg_ext[:, :], in_=gd[:, :])
                continue
            for j in range(kk):
                t = t0 + j
                w = win_of_tile[t]
                slot = w % 4
                i = w // 4
                if slot == 0 and win_start[w] == t:
                    psum_cur[0] = pools["ps"].tile([P, 512], f32, space="PSUM",
                                                   tag="agg_ps", name="agg_ps")
                ps = psum_cur[0]
                nc.tensor.matmul(
                    out=ps[32 * slot:32 * (slot + 1), 0:hhc],
                    lhsT=St[:, j * W:(j + 1) * W], rhs=hh[:, j, :],
                    start=(win_start[w] == t), stop=(win_end[w] == t),
                    tile_position=(0, 32 * slot))
                if slot == 3 and win_end[w] == t:
                    post_chunk(pools, l, i, ps)
                    after_chunk(i)

    def mk_pools(tc, stk):
        return dict(
            sb=stk.enter_context(tc.tile_pool(name="sb", bufs=3)),
            gb=stk.enter_context(tc.tile_pool(name="gb", bufs=2)),
            ps=stk.enter_context(tc.tile_pool(name="ps", bufs=2, space="PSUM")),
        )

    def collective(kind, op, in_dram, out_dram, sem):
        nc.gpsimd.collective_compute(
            kind, op, ins=[in_dram[:, :].opt()], outs=[out_dram[:, :].opt()],
            replica_groups=[list(range(NC))]).then_inc(sem, 1)
        nc.gpsimd.wait_ge(sem, 1)
        # other engines must not enter the next TileContext before the
        # collective's output is visible
        nc.all_engine_barrier()

    # ---- TC0: constants + Wext(all layers) + dense layer 1 ----
    with tile.TileContext(nc) as tc, ExitStack() as stk:
        pools = mk_pools(tc, stk)
        make_identity(nc, ident[:, :])
        nc.vector.memset(ones_row[:, :], 1.0)
        nc.vector.memset(hfext[:, :, 64:65], 1.0)
        nc.sync.dma_start(out=lin1e[0:64, :], in_=l1w_in[:, :])
        nc.sync.dma_start(out=lin1e[64:65, :], in_=l1b_in[:, :])
        nc.sync.dma_start(out=lin2e[0:32, :], in_=l2w_in[:, :])
        nc.sync.dma_start(out=lin2e[32:33, :], in_=l2b_in[:, :])
        for l in (1, 2, 3):
            build_wext(pools, l)
        for i in range(NCHK):
            dense_chunk(pools, 1, i, from_dram=True)
    def dump_dram(src_dram, rows, cols):
        with tile.TileContext(nc) as tc, ExitStack() as stk:
            pools = mk_pools(tc, stk)
            d = pools["sb"].tile([P, ROW1], f32, name="d")
            nc.vector.memset(d[:, :], 0.0)
            dd = pools["sb"].tile([P, ROW1], tdt, name="dd")
            nc.sync.dma_start(out=dd[0:rows, 0:cols], in_=src_dram)
            nc.vector.tensor_copy(out=d[0:rows, 0:cols], in_=dd[0:rows, 0:cols])
            nc.sync.dma_start(out=dbg_ext[:, :], in_=d[:, :])

    def dump_sbuf(src_ap, rows, cols):
        import os as _os
        if _os.environ.get("KERNEL_EDGE_MODE", "full") != "full":
            return
        with tile.TileContext(nc) as tc, ExitStack() as stk:
            pools = mk_pools(tc, stk)
            d = pools["sb"].tile([P, ROW1], f32, name="d")
            nc.vector.memset(d[:, :], 0.0)
            nc.vector.tensor_copy(out=d[0:rows, 0:cols], in_=src_ap)
            nc.sync.dma_start(out=dbg_ext[:, :], in_=d[:, :])

    if phases == 0:
        dump_dram(shard12[0:P, 0:ROW1], P, ROW1)
        nc.compile()
        es.close()
        return nc
    collective("AllGather", AT.bypass, shard12, table12, cc_sems[0])
    if phases == 1:
        dump_dram(table12[3 * SHN:3 * SHN + P, 0:ROW1], P, ROW1)
        nc.compile()
        es.close()
        return nc

    # ---- TC1: edges layer 1 + dense layer 2 ----
    with tile.TileContext(nc) as tc, ExitStack() as stk:
        pools = mk_pools(tc, stk)
        edge_phase(pools, 1, lambda i: dense_chunk(pools, 2, i))
    if phases == 2:
        dump_sbuf(xnext[:, 0, 0:256], P, 256)
        nc.compile()
        es.close()
        return nc
    collective("AllGather", AT.bypass, shard12, table12, cc_sems[1])

    # ---- TC2: edges layer 2 + dense layer 3 ----
    with tile.TileContext(nc) as tc, ExitStack() as stk:
        pools = mk_pools(tc, stk)
        edge_phase(pools, 2, lambda i: dense_chunk(pools, 3, i))
    if phases == 3:
        dump_sbuf(xnext[:, 0, 0:256], P, 256)
        nc.compile()
        es.close()
        return nc
    collective("AllGather", AT.bypass, shard3, table3, cc_sems[2])

    # ---- TC3: edges layer 3 + readout partials ----
    with tile.TileContext(nc) as tc, ExitStack() as stk:
        pools = mk_pools(tc, stk)
        ro_ps = pools["ps"].tile([NGR, HID + 1], f32, space="PSUM", tag="ro_ps")

        def ro_chunk(i):
            Sbt = pools["sb"].tile([P, NGR], f32, tag="Sbt")
            nc.sync.dma_start(out=Sbt[:, :], in_=Sb_in[i, :, :])
            nc.tensor.matmul(out=ro_ps[:], lhsT=Sbt[:, :], rhs=hfext[:, i, :],
                             start=(i == 0), stop=(i == NCHK - 1))
            if i == NCHK - 1:
                ro_sb = pools["sb"].tile([NGR, HID + 1], f32, tag="ro_sb")
                nc.vector.tensor_copy(out=ro_sb[:], in_=ro_ps[:])
                nc.sync.dma_start(out=ro_in[:, :], in_=ro_sb[:])

        edge_phase(pools, 3, ro_chunk)
    if phases == 4:
        dump_dram(ro_in[:, :], NGR, HID + 1)
        nc.compile()
        es.close()
        return nc
    collective("AllReduce", AT.add, ro_in, ro_out, cc_sems[3])

    # ---- TC4: final MLP ----
    with tile.TileContext(nc) as tc, ExitStack() as stk:
        pools = mk_pools(tc, stk)
        ro = pools["sb"].tile([NGR, HID + 1], f32)
        nc.sync.dma_start(out=ro[:], in_=ro_out[:, :])
        cnt = pools["sb"].tile([NGR, 1], f32)
        nc.vector.tensor_scalar_max(cnt[:], ro[:, 64:65], 1.0)
        rec = pools["sb"].tile([NGR, 1], f32)
        nc.vector.reciprocal(rec[:], cnt[:])
        gext = pools["sb"].tile([NGR, HID + 1], f32)
        nc.vector.tensor_scalar_mul(gext[:, 0:64], ro[:, 0:64], rec[:, 0:1])
        nc.vector.memset(gext[:, 64:65], 1.0)
        gT_ps = pools["ps"].tile([HID + 1, NGR], f32, space="PSUM")
        nc.tensor.transpose(out=gT_ps[:], in_=gext[:], identity=ident[0:NGR, 0:NGR])
        gT = pools["sb"].tile([HID + 1, NGR], f32)
        nc.vector.tensor_copy(out=gT[:], in_=gT_ps[:])
        o1_ps = pools["ps"].tile([NGR, 32], f32, space="PSUM")
        nc.tensor.matmul(out=o1_ps[:], lhsT=gT[:], rhs=lin1e[:, :],
                         start=True, stop=True)
        o1e = pools["sb"].tile([NGR, 33], f32)
        nc.vector.tensor_relu(out=o1e[:, 0:32], in_=o1_ps[:])
        nc.vector.memset(o1e[:, 32:33], 1.0)
        o1T_ps = pools["ps"].tile([33, NGR], f32, space="PSUM")
        nc.tensor.transpose(out=o1T_ps[:], in_=o1e[:], identity=ident[0:NGR, 0:NGR])
        o1T = pools["sb"].tile([33, NGR], f32)
        nc.vector.tensor_copy(out=o1T[:], in_=o1T_ps[:])
        o2_ps = pools["ps"].tile([NGR, NCLS], f32, space="PSUM")
        nc.tensor.matmul(out=o2_ps[:], lhsT=o1T[:], rhs=lin2e[:, :],
                         start=True, stop=True)
        o2 = pools["sb"].tile([NGR, NCLS], f32)
        nc.vector.tensor_copy(out=o2[:], in_=o2_ps[:])
        nc.sync.dma_start(out=out_ext[:, :], in_=o2[:])

    nc.compile()
    es.close()
    return nc


def _bf16():
    from ml_dtypes import bfloat16
    return bfloat16


_CACHE = {}


def _get_built(ei, batch):
    import os
    if "k" not in _CACHE:
        core_arrays, win_of_tile, tiles_per_win, nch = _preprocess(ei)
        SbL = _batch_S(batch)
        nc = _build(win_of_tile, nch, phases=int(os.environ.get("KERNEL_PHASES", "5")))
        _CACHE["k"] = (nc, core_arrays, SbL)
    return _CACHE["k"]


def _in_maps(inputs, core_arrays, SbL):
    x = np.ascontiguousarray(np.asarray(inputs["x"], np.float32))
    maps = []
    for c in range(NC):
        m = dict(
            x_sh=x[c * SHN:(c + 1) * SHN],
            W1=np.asarray(inputs["W1"], np.float32),
            W2=np.asarray(inputs["W2"], np.float32),
            W3=np.asarray(inputs["W3"], np.float32),
            a1s=np.asarray(inputs["a1_src"], np.float32).reshape(1, 256),
            a1d=np.asarray(inputs["a1_dst"], np.float32).reshape(1, 256),
            a2s=np.asarray(inputs["a2_src"], np.float32).reshape(1, 256),
            a2d=np.asarray(inputs["a2_dst"], np.float32).reshape(1, 256),
            a3s=np.asarray(inputs["a3_src"], np.float32).reshape(1, 64),
            a3d=np.asarray(inputs["a3_dst"], np.float32).reshape(1, 64),
            b3=np.asarray(inputs["b3"], np.float32).reshape(1, 64),
            lin1_w=np.asarray(inputs["lin1_w"], np.float32),
            lin1_b=np.asarray(inputs["lin1_b"], np.float32).reshape(1, 32),
            lin2_w=np.asarray(inputs["lin2_w"], np.float32),
            lin2_b=np.asarray(inputs["lin2_b"], np.float32).reshape(1, 5),
            si=core_arrays[c]["si"],
            di=core_arrays[c]["di"],
            S=(core_arrays[c]["S"].astype(_bf16()) if BF16_TABLE
               else core_arrays[c]["S"]),
            Sb=SbL[c],
        )
        for nm in ("b1", "b2", "bn1_gamma", "bn1_beta", "bn1_mean", "bn1_var",
                   "bn2_gamma", "bn2_beta", "bn2_mean", "bn2_var"):
            m[nm] = np.asarray(inputs[nm], np.float32).reshape(1, 256)
        maps.append(m)
    return maps


def kernel(**inputs):
    from concourse.bass_utils import run_bass_kernel_spmd
    ei = np.asarray(inputs["edge_index"])
    batch = np.asarray(inputs["batch"])
    nc, core_arrays, SbL = _get_built(ei, batch)
    maps = _in_maps(inputs, core_arrays, SbL)
    res = run_bass_kernel_spmd(nc, maps, core_ids=list(range(NC)))
    return np.asarray(res.results[0]["out"], np.float32)
